# revision 20
# baseline (speedup 1.0000x reference)
"""Trainium2 Bass kernel for masked dot-product-attention-with-distance.

Computes, for each batch b:
    raw    = Q @ K^T - 0.5*||k||^2          [Q, K]
    scaled = (raw + d/2) / sqrt(3d/2)
    masked softmax over k (k < valid_len[b, q]), then weights @ V.

Strategy:
  - Data-parallel over batch: 8 cores x 2 batches each.
  - Host: per batch, sort q rows by valid_len; pass Q^T and K^T layouts; fold
    the (d/2 - 0.5||k||^2)/sqrt(3d/2) term into a per-key bias applied on the
    ACT engine (exp(scale*S + bias)); precompute boundary 0/1 masks.
  - Device: S^T tiles [kpos=128, q<=512] via PE matmul (lhsT=K^T slice,
    rhs=Q^T slice); exp on ACT straight out of PSUM (scores are bounded, no
    max subtraction needed -- softmax is shift-invariant and exp stays in
    fp32 range); boundary masks multiplied in on DVE; O^T = V^T-stationary
    matmul accumulation over kpos tiles; denominator via ones-matmul;
    normalize with DVE reciprocal + PE transpose; DMA out.
  - Because q rows are sorted by valid_len, per (q-chunk, kpos-tile) ranges
    are trimmed at compile time (the program is specialized to the actual
    valid_lens): fully-masked regions are never computed and only boundary
    tiles pay masking cost.
"""

import math
import os
import time

import numpy as np

B, Q, K, D, DV = 16, 2048, 2048, 128, 128
N_CORES = 8
BPC = B // N_CORES  # batches per core (slots)
QCH = 512  # q chunk width (moving operand / PSUM bank)
NJ = Q // QCH  # 4
KT = 128  # kpos tile (contraction partition dim)
NKT = K // KT  # 16
ALPHA = float(1.0 / math.sqrt(3.0 * D / 2.0))

LAST_EXEC_NS = None
LAST_WALL_S = None
LAST_RESULTS = None

_program_cache = {}


def _compute_structure(Ls_by_slot):
    """Ls_by_slot[s] : [n_batches, Q] sorted valid_lens (ascending) for the
    batches mapped to slot s.  Returns per-slot compile-time structure:

    struct[s][j] = list of (c, st, width, m_lo, m_w, is_last) with
      st    : within-chunk q column where the matmul range starts (mult of 4)
      width : matmul free size = QCH - st
      m_lo  : mask window start (== st), m_w: mask window width (0 = no mask)
    """
    struct = []
    for s in range(BPC):
        Ls = Ls_by_slot[s]
        per_j = []
        for j in range(NJ):
            chunks = Ls[:, j * QCH : (j + 1) * QCH]  # [nb, QCH] sorted asc
            entries = []
            hot_cs = []
            for c in range(NKT):
                lo_key = c * KT  # L <= lo_key  -> tile c fully invalid
                hi_key = c * KT + KT - 1  # L <= hi_key -> needs masking
                qstart = int(
                    min(np.searchsorted(chunks[b], lo_key, side="right")
                        for b in range(chunks.shape[0]))
                )
                if qstart >= QCH:
                    break  # start is nondecreasing in c -> all later c skipped
                mend = int(
                    max(np.searchsorted(chunks[b], hi_key, side="right")
                        for b in range(chunks.shape[0]))
                )
                st = qstart & ~3  # align to 16B for PSUM-friendly APs
                m_hi = max(mend, qstart)
                m_w = m_hi - st if m_hi > st else 0
                hot_cs.append((c, st, QCH - st, st, m_w))
            for idx, (c, st, width, m_lo, m_w) in enumerate(hot_cs):
                entries.append((c, st, width, m_lo, m_w, idx == len(hot_cs) - 1))
            per_j.append(entries)
        struct.append(per_j)
    return struct


def _build_masks(struct, Ls_by_core_slot):
    """Lay out mask windows in a flat column blob (shared offsets across
    cores); returns (offsets dict {(s,j,c): (off,w)}, total_w, masks array
    [n_cores, BPC, 128, total_w] float32)."""
    offsets = {}
    off = 0
    for s in range(BPC):
        for j in range(NJ):
            for (c, st, width, m_lo, m_w, last) in struct[s][j]:
                if m_w > 0:
                    offsets[(s, j, c)] = (off, m_w)
                    off += m_w
    total_w = max(off, 4)
    # additive masks applied to raw scores pre-exp: 0.0 = valid,
    # -1e5 = invalid (exp underflows to exactly 0 after *ALPHA scaling)
    masks = np.zeros((N_CORES, BPC, 128, total_w), dtype=np.float32)
    kpos_col = np.arange(128, dtype=np.int64)[:, None]
    for (s, j, c), (o, w) in offsets.items():
        for n in range(N_CORES):
            Ls = Ls_by_core_slot[n][s]
            st = None
            for (cc, st_, width, m_lo, m_w, last) in struct[s][j]:
                if cc == c:
                    st = m_lo
                    break
            colL = Ls[j * QCH + st : j * QCH + st + w][None, :]  # [1, w]
            masks[n, s, :, o : o + w] = np.where(
                (kpos_col + c * KT) < colL, 0.0, -1e5
            ).astype(np.float32)
    return offsets, total_w, masks


def _build_program(struct, offsets, total_w):
    import concourse.bass as bass
    import concourse.bacc as bacc
    import concourse.mybir as mybir
    import concourse.tile as tile

    f32 = mybir.dt.float32
    f32r = mybir.dt.float32r  # single-pass PE matmul (full rate at N>=256)
    nc = bacc.Bacc("TRN2", target_bir_lowering=False, debug=False,
                   num_devices=N_CORES)

    qt_d = nc.dram_tensor("qt", [BPC, D, Q], f32r, kind="ExternalInput")
    kt_d = nc.dram_tensor("kt", [BPC, D, K], f32r, kind="ExternalInput")
    v_d = nc.dram_tensor("vp", [BPC, 128, NKT * DV], f32r, kind="ExternalInput")
    bias_d = nc.dram_tensor("bias", [BPC, 128, NKT], f32, kind="ExternalInput")
    mask_d = nc.dram_tensor("masks", [BPC, 128, total_w], f32,
                            kind="ExternalInput")
    ones_d = nc.dram_tensor("ones", [128, 1], f32r, kind="ExternalInput")
    out_d = nc.dram_tensor("out", [BPC, 128, (Q // 128) * DV], f32,
                           kind="ExternalOutput")

    max_mw = max([w for (_, w) in offsets.values()] + [4])

    with tile.TileContext(nc) as tc:
        with (
            tc.tile_pool(name="pin", bufs=2) as pin,
            tc.tile_pool(name="pconst", bufs=1) as pconst,
            tc.tile_pool(name="pp", bufs=4) as pp,
            tc.tile_pool(name="pm", bufs=4) as pm,
            tc.tile_pool(name="pacc", bufs=3) as pacc,
            tc.tile_pool(name="pout", bufs=3) as pout,
            tc.tile_pool(name="psum_s", bufs=3, space="PSUM") as psum_s,
            tc.tile_pool(name="psum_o", bufs=2, space="PSUM") as psum_o,
            tc.tile_pool(name="psum_d", bufs=2, space="PSUM") as psum_d,
            tc.tile_pool(name="psum_t", bufs=1, space="PSUM") as psum_t,
        ):
            ones_sb = pconst.tile([128, 1], f32r)
            nc.sync.dma_start(out=ones_sb, in_=ones_d.ap())
            ident_sb = pconst.tile([128, 128], f32)
            from concourse.masks import make_identity
            make_identity(nc, ident_sb)
            # warm the ACT exp table set before real work arrives
            warm_in = pconst.tile([128, 1], f32)
            nc.vector.memset(warm_in, 0.0)
            warm_out = pconst.tile([128, 1], f32)
            nc.scalar.activation(warm_out, warm_in,
                                 mybir.ActivationFunctionType.Exp)

            def make_epilogue(s, j, otj_sb, den_row):
                def emit():
                    den_cols = psum_d.tile([128, QCH // 128], f32, tag="den",
                                           name="den_cols")
                    for t in range(QCH // 128):
                        nc.tensor.matmul(
                            den_cols[:, t : t + 1],
                            lhsT=den_row[0:1, bass.ts(t, 128)],
                            rhs=ident_sb[0:1, 0:1],
                            start=True, stop=True,
                        )
                    recip_sb = pacc.tile([128, QCH // 128], f32,
                                         name="recip_sb")
                    nc.vector.reciprocal(recip_sb, den_cols)
                    o_ps = psum_t.tile([128, QCH], f32, name="o_ps")
                    for t in range(QCH // 128):
                        nc.tensor.transpose(
                            o_ps[:, bass.ts(t, 128)],
                            otj_sb[:, bass.ts(t, 128)], ident_sb,
                        )
                    out_sb = pout.tile([128, QCH // 128, DV], f32,
                                       name="out_sb")
                    for t in range(QCH // 128):
                        nc.vector.tensor_scalar_mul(
                            out_sb[:, t, :], in0=o_ps[:, bass.ts(t, 128)],
                            scalar1=recip_sb[:, t : t + 1],
                        )
                    nc.sync.dma_start(
                        out=out_d.ap()[s][:, j * QCH : (j + 1) * QCH]
                            .rearrange("p (t d) -> p t d", d=DV),
                        in_=out_sb,
                    )
                return emit

            pending_epilogue = None
            for s in range(BPC):
                bias_sb = pin.tile([128, NKT], f32)
                nc.sync.dma_start(out=bias_sb, in_=bias_d.ap()[s])
                kt_sb = pin.tile([128, K], f32r)
                qt_sb = pin.tile([128, Q], f32r)
                v_sb = pin.tile([128, NKT * DV], f32r)
                nc.sync.dma_start(out=kt_sb[:, bass.ts(0, 128)],
                                  in_=kt_d.ap()[s][:, bass.ts(0, 128)])
                nc.sync.dma_start(out=qt_sb[:, bass.ts(0, 512)],
                                  in_=qt_d.ap()[s][:, bass.ts(0, 512)])
                nc.sync.dma_start(out=kt_sb[:, 128:512],
                                  in_=kt_d.ap()[s][:, 128:512])
                for g in range(1, 4):
                    sl = bass.ts(g, 512)
                    nc.sync.dma_start(out=kt_sb[:, sl], in_=kt_d.ap()[s][:, sl])
                    nc.sync.dma_start(out=qt_sb[:, sl], in_=qt_d.ap()[s][:, sl])
                for g in range(4):
                    sl = bass.ts(g, 512)
                    nc.sync.dma_start(out=v_sb[:, sl], in_=v_d.ap()[s][:, sl])

                for j in range(NJ):
                    ot_ps = psum_o.tile([128, QCH], f32)
                    den_ps = psum_d.tile([1, QCH], f32, tag="den")
                    for idx, (c, st, width, m_lo, m_w, is_last) in enumerate(
                            struct[s][j]):
                        s_ps = psum_s.tile([128, QCH], f32, tag="s_ps")
                        nc.tensor.matmul(
                            s_ps[:, :width],
                            lhsT=kt_sb[:, bass.ts(c, KT)],
                            rhs=qt_sb[:, j * QCH + st : (j + 1) * QCH],
                            start=True, stop=True,
                        )
                        if m_w > 0:
                            off, w = offsets[(s, j, c)]
                            m_sb = pm.tile([128, max_mw], f32)
                            nc.gpsimd.dma_start(
                                out=m_sb[:, :w],
                                in_=mask_d.ap()[s][:, off : off + w],
                            )
                            nc.vector.tensor_add(
                                s_ps[:, : m_w],
                                s_ps[:, : m_w],
                                m_sb[:, :w],
                            )
                        p_sb = pp.tile([128, QCH], f32r)
                        nc.scalar.activation(
                            p_sb[:, :width],
                            s_ps[:, :width],
                            mybir.ActivationFunctionType.Exp,
                            bias=bias_sb[:, c : c + 1],
                            scale=ALPHA,
                        )
                        nc.tensor.matmul(
                            ot_ps[:, st:],
                            lhsT=v_sb[:, bass.ts(c, DV)],
                            rhs=p_sb[:, :width],
                            start=(c == 0), stop=is_last,
                        )
                        nc.tensor.matmul(
                            den_ps[:, st:],
                            lhsT=ones_sb,
                            rhs=p_sb[:, :width],
                            start=(c == 0), stop=is_last,
                        )
                        if idx == 2 and pending_epilogue is not None:
                            pending_epilogue()
                            pending_epilogue = None
                    # evacuate O^T and denominators; heavy epilogue is
                    # deferred into the next chunk's stream
                    otj_sb = pacc.tile([128, QCH], f32)
                    nc.vector.tensor_copy(otj_sb, ot_ps)
                    den_row = pacc.tile([1, QCH], f32)
                    nc.vector.tensor_copy(den_row, den_ps)
                    if pending_epilogue is not None:
                        pending_epilogue()
                    pending_epilogue = make_epilogue(s, j, otj_sb, den_row)
            pending_epilogue()
    nc.compile()
    return nc


def _prepare(queries, keys, values, valid_lens):
    """Host-side prep. Returns (key_sig, struct, offsets, total_w, in_maps,
    sortidx)."""
    queries = np.ascontiguousarray(np.asarray(queries, dtype=np.float32))
    keys = np.ascontiguousarray(np.asarray(keys, dtype=np.float32))
    values = np.ascontiguousarray(np.asarray(values, dtype=np.float32))
    vl = np.asarray(valid_lens, dtype=np.int64)

    # ---- host prep: per-batch sort by valid_len --------------------------
    sortidx = np.argsort(vl, axis=1, kind="stable")  # [B, Q]
    Ls = np.take_along_axis(vl, sortidx, axis=1)  # [B, Q] ascending

    # slot s of core n holds batch 2n + s
    Ls_by_slot = [Ls[s::BPC] for s in range(BPC)]  # each [8, Q]
    struct = _compute_structure(Ls_by_slot)
    Ls_by_core_slot = [[Ls[n * BPC + s] for s in range(BPC)]
                       for n in range(N_CORES)]
    offsets, total_w, masks = _build_masks(struct, Ls_by_core_slot)

    key_sig = (total_w, tuple(
        (s, j, c, st, width, m_lo, m_w, last)
        for s in range(BPC) for j in range(NJ)
        for (c, st, width, m_lo, m_w, last) in struct[s][j]
    ))

    # ---- per-core input maps --------------------------------------------
    biases = (D / 2.0 - 0.5 * (keys.astype(np.float64) ** 2).sum(-1)) * ALPHA
    biases = biases.astype(np.float32)  # [B, K]

    in_maps = []
    for n in range(N_CORES):
        qt = np.empty((BPC, D, Q), np.float32)
        kt = np.empty((BPC, D, K), np.float32)
        vp = np.empty((BPC, 128, NKT * DV), np.float32)
        bias_arr = np.empty((BPC, 128, NKT), np.float32)
        for s in range(BPC):
            b = n * BPC + s
            qt[s] = queries[b][sortidx[b]].T
            kt[s] = keys[b].T
            vp[s] = (values[b].reshape(NKT, 128, DV)
                     .transpose(1, 0, 2).reshape(128, NKT * DV))
            bias_arr[s] = biases[b].reshape(NKT, 128).T
        in_maps.append({
            "qt": qt, "kt": kt, "vp": vp, "bias": bias_arr,
            "masks": np.ascontiguousarray(masks[n]),
            "ones": np.ones((128, 1), np.float32),
        })
    return key_sig, struct, offsets, total_w, in_maps, sortidx


def get_program(key_sig, struct, offsets, total_w):
    if key_sig not in _program_cache:
        _program_cache.clear()
        _program_cache[key_sig] = _build_program(struct, offsets, total_w)
    return _program_cache[key_sig]


def kernel(queries, keys, values, valid_lens):
    global LAST_EXEC_NS, LAST_WALL_S, LAST_RESULTS
    key_sig, struct, offsets, total_w, in_maps, sortidx = _prepare(
        queries, keys, values, valid_lens
    )
    nc = get_program(key_sig, struct, offsets, total_w)

    # ---- run on 8 cores --------------------------------------------------
    from concourse.bass_utils import run_bass_kernel_spmd

    trace = bool(int(os.environ.get("KBENCH_TRACE", "0")))
    kwargs = {}
    tdir = os.environ.get("KBENCH_TRACE_DIR")
    if trace and tdir:
        kwargs["tmpdir"] = tdir
    t0 = time.perf_counter()
    try:
        res = run_bass_kernel_spmd(
            nc, in_maps, core_ids=list(range(N_CORES)), trace=trace, **kwargs
        )
    except Exception:
        if not trace:
            raise
        import traceback
        traceback.print_exc()
        res = run_bass_kernel_spmd(
            nc, in_maps, core_ids=list(range(N_CORES)), trace=False
        )
    LAST_WALL_S = time.perf_counter() - t0
    LAST_EXEC_NS = res.exec_time_ns
    LAST_RESULTS = res

    # ---- gather + undo the sort -----------------------------------------
    out = np.empty((B, Q, DV), dtype=np.float32)
    for n in range(N_CORES):
        o = res.results[n]["out"]  # [BPC, 128, (Q//128)*DV]
        for s in range(BPC):
            b = n * BPC + s
            # device rows: q_sorted = 128*t + p  ->  [p, t, d]
            osort = (o[s].reshape(128, Q // 128, DV)
                     .transpose(1, 0, 2).reshape(Q, DV))
            out[b][sortidx[b]] = osort
    return out


# revision 24
# speedup vs baseline: 1.0775x; 1.0775x over previous
"""Trainium2 Bass kernel for masked dot-product-attention-with-distance.

Computes, for each batch b:
    raw    = Q @ K^T - 0.5*||k||^2          [Q, K]
    scaled = (raw + d/2) / sqrt(3d/2)
    masked softmax over k (k < valid_len[b, q]), then weights @ V.

Strategy:
  - Data-parallel over batch: 8 cores x 2 batches each.
  - Host: per batch, sort q rows by valid_len; pass Q^T and K^T layouts; fold
    the (d/2 - 0.5||k||^2)/sqrt(3d/2) term into a per-key bias applied on the
    ACT engine (exp(scale*S + bias)); precompute boundary 0/1 masks.
  - Device: S^T tiles [kpos=128, q<=512] via PE matmul (lhsT=K^T slice,
    rhs=Q^T slice); exp on ACT straight out of PSUM (scores are bounded, no
    max subtraction needed -- softmax is shift-invariant and exp stays in
    fp32 range); boundary masks multiplied in on DVE; O^T = V^T-stationary
    matmul accumulation over kpos tiles; denominator via ones-matmul;
    normalize with DVE reciprocal + PE transpose; DMA out.
  - Because q rows are sorted by valid_len, per (q-chunk, kpos-tile) ranges
    are trimmed at compile time (the program is specialized to the actual
    valid_lens): fully-masked regions are never computed and only boundary
    tiles pay masking cost.
"""

import math
import os
import time

import numpy as np

B, Q, K, D, DV = 16, 2048, 2048, 128, 128
N_CORES = 8
BPC = B // N_CORES  # batches per core (slots)
QCH = 512  # q chunk width (moving operand / PSUM bank)
NJ = Q // QCH  # 4
KT = 128  # kpos tile (contraction partition dim)
NKT = K // KT  # 16
ALPHA = float(1.0 / math.sqrt(3.0 * D / 2.0))

LAST_EXEC_NS = None
LAST_WALL_S = None
LAST_RESULTS = None

_program_cache = {}


def _compute_structure(Ls_by_slot):
    """Ls_by_slot[s] : [n_batches, Q] sorted valid_lens (ascending) for the
    batches mapped to slot s.  Returns per-slot compile-time structure:

    struct[s][j] = list of (c, st, width, m_lo, m_w, is_last) with
      st    : within-chunk q column where the matmul range starts (mult of 4)
      width : matmul free size = QCH - st
      m_lo  : mask window start (== st), m_w: mask window width (0 = no mask)
    """
    struct = []
    for s in range(BPC):
        Ls = Ls_by_slot[s]
        per_j = []
        for j in range(NJ):
            chunks = Ls[:, j * QCH : (j + 1) * QCH]  # [nb, QCH] sorted asc
            entries = []
            hot_cs = []
            for c in range(NKT):
                lo_key = c * KT  # L <= lo_key  -> tile c fully invalid
                hi_key = c * KT + KT - 1  # L <= hi_key -> needs masking
                qstart = int(
                    min(np.searchsorted(chunks[b], lo_key, side="right")
                        for b in range(chunks.shape[0]))
                )
                if qstart >= QCH:
                    break  # start is nondecreasing in c -> all later c skipped
                mend = int(
                    max(np.searchsorted(chunks[b], hi_key, side="right")
                        for b in range(chunks.shape[0]))
                )
                st = qstart & ~3  # align to 16B for PSUM-friendly APs
                m_hi = max(mend, qstart)
                m_w = m_hi - st if m_hi > st else 0
                hot_cs.append((c, st, QCH - st, st, m_w))
            for idx, (c, st, width, m_lo, m_w) in enumerate(hot_cs):
                entries.append((c, st, width, m_lo, m_w, idx == len(hot_cs) - 1))
            per_j.append(entries)
        struct.append(per_j)
    return struct


def _build_masks(struct, Ls_by_core_slot):
    """Lay out mask windows in a flat column blob (shared offsets across
    cores); returns (offsets dict {(s,j,c): (off,w)}, total_w, masks array
    [n_cores, BPC, 128, total_w] float32)."""
    offsets = {}
    off = 0
    for s in range(BPC):
        for j in range(NJ):
            for (c, st, width, m_lo, m_w, last) in struct[s][j]:
                if m_w > 0:
                    offsets[(s, j, c)] = (off, m_w)
                    off += m_w
    total_w = max(off, 4)
    # additive masks applied to raw scores pre-exp: 0.0 = valid,
    # -1e5 = invalid (exp underflows to exactly 0 after *ALPHA scaling)
    masks = np.zeros((N_CORES, BPC, 128, total_w), dtype=np.float32)
    kpos_col = np.arange(128, dtype=np.int64)[:, None]
    for (s, j, c), (o, w) in offsets.items():
        for n in range(N_CORES):
            Ls = Ls_by_core_slot[n][s]
            st = None
            for (cc, st_, width, m_lo, m_w, last) in struct[s][j]:
                if cc == c:
                    st = m_lo
                    break
            colL = Ls[j * QCH + st : j * QCH + st + w][None, :]  # [1, w]
            masks[n, s, :, o : o + w] = np.where(
                (kpos_col + c * KT) < colL, 0.0, -1e5
            ).astype(np.float32)
    return offsets, total_w, masks


def _build_program(struct, offsets, total_w):
    import concourse.bass as bass
    import concourse.bacc as bacc
    import concourse.mybir as mybir
    import concourse.tile as tile

    f32 = mybir.dt.float32
    f32r = mybir.dt.float32r  # single-pass PE matmul (full rate at N>=256)
    nc = bacc.Bacc("TRN2", target_bir_lowering=False, debug=False,
                   num_devices=N_CORES)

    qt_d = nc.dram_tensor("qt", [BPC, D, Q], f32r, kind="ExternalInput")
    kt_d = nc.dram_tensor("kt", [BPC, D, K], f32r, kind="ExternalInput")
    v_d = nc.dram_tensor("vp", [BPC, 128, NKT * DV], f32r, kind="ExternalInput")
    bias_d = nc.dram_tensor("bias", [BPC, 128, NKT], f32, kind="ExternalInput")
    mask_d = nc.dram_tensor("masks", [BPC, 128, total_w], f32,
                            kind="ExternalInput")
    ones_d = nc.dram_tensor("ones", [128, 129], f32r, kind="ExternalInput")
    out_d = nc.dram_tensor("out", [BPC, 128, (Q // 128) * DV], f32,
                           kind="ExternalOutput")

    max_mw = max([w for (_, w) in offsets.values()] + [4])

    with tile.TileContext(nc) as tc:
        with (
            tc.tile_pool(name="pin", bufs=2) as pin,
            tc.tile_pool(name="pconst", bufs=1) as pconst,
            tc.tile_pool(name="pp", bufs=5) as pp,
            tc.tile_pool(name="pm", bufs=4) as pm,
            tc.tile_pool(name="pacc", bufs=3) as pacc,
            tc.tile_pool(name="pout", bufs=3) as pout,
            tc.tile_pool(name="psum_s", bufs=3, space="PSUM") as psum_s,
            tc.tile_pool(name="psum_o", bufs=2, space="PSUM") as psum_o,
            tc.tile_pool(name="psum_d", bufs=2, space="PSUM") as psum_d,
            tc.tile_pool(name="psum_t", bufs=1, space="PSUM") as psum_t,
        ):
            ones_sb = pconst.tile([128, 129], f32r)
            nc.sync.dma_start(out=ones_sb, in_=ones_d.ap())
            ones_f = pconst.tile([128, 1], f32)
            nc.vector.memset(ones_f, 1.0)
            ident_sb = pconst.tile([128, 128], f32)
            from concourse.masks import make_identity
            make_identity(nc, ident_sb)
            # warm the ACT exp table set before real work arrives
            warm_in = pconst.tile([128, 1], f32)
            nc.vector.memset(warm_in, 0.0)
            warm_out = pconst.tile([128, 1], f32)
            nc.scalar.activation(warm_out, warm_in,
                                 mybir.ActivationFunctionType.Exp)

            def make_epilogue(s, j, otj_sb, den_row):
                def emit():
                    den_cols = psum_d.tile([128, QCH // 128], f32, tag="den",
                                           name="den_cols")
                    for t in range(QCH // 128):
                        nc.tensor.matmul(
                            den_cols[:, t : t + 1],
                            lhsT=den_row[0:1, bass.ts(t, 128)],
                            rhs=ones_f[0:1, 0:1],
                            start=True, stop=True,
                        )
                    recip_sb = pacc.tile([128, QCH // 128], f32,
                                         name="recip_sb")
                    nc.vector.reciprocal(recip_sb, den_cols)
                    o_ps = psum_t.tile([128, QCH], f32, name="o_ps")
                    for t in range(QCH // 128):
                        nc.tensor.transpose(
                            o_ps[:, bass.ts(t, 128)],
                            otj_sb[:, bass.ts(t, 128)], ident_sb,
                        )
                    out_sb = pout.tile([128, QCH // 128, DV], f32,
                                       name="out_sb")
                    for t in range(QCH // 128):
                        nc.vector.tensor_scalar_mul(
                            out_sb[:, t, :], in0=o_ps[:, bass.ts(t, 128)],
                            scalar1=recip_sb[:, t : t + 1],
                        )
                    nc.sync.dma_start(
                        out=out_d.ap()[s][:, j * QCH : (j + 1) * QCH]
                            .rearrange("p (t d) -> p t d", d=DV),
                        in_=out_sb,
                    )
                return emit

            pending_epilogue = None
            for s in range(BPC):
                bias_sb = pin.tile([128, NKT], f32)
                nc.sync.dma_start(out=bias_sb, in_=bias_d.ap()[s])
                kt_sb = pin.tile([128, K], f32r)
                qt_sb = pin.tile([128, Q], f32r)
                v_sb = pin.tile([128, NKT * DV], f32r)
                nc.sync.dma_start(out=kt_sb[:, 0:512],
                                  in_=kt_d.ap()[s][:, 0:512])
                nc.sync.dma_start(out=qt_sb[:, 0:512],
                                  in_=qt_d.ap()[s][:, 0:512])
                nc.sync.dma_start(out=v_sb[:, 0:512],
                                  in_=v_d.ap()[s][:, 0:512])
                nc.sync.dma_start(out=kt_sb[:, 512:K],
                                  in_=kt_d.ap()[s][:, 512:K])
                nc.sync.dma_start(out=qt_sb[:, 512:Q],
                                  in_=qt_d.ap()[s][:, 512:Q])
                nc.sync.dma_start(out=v_sb[:, 512:NKT * DV],
                                  in_=v_d.ap()[s][:, 512:NKT * DV])

                for j in range(NJ):
                    ot_ps = psum_o.tile([128, QCH], f32)
                    den_ps = psum_d.tile([1, QCH], f32, tag="den")
                    for idx, (c, st, width, m_lo, m_w, is_last) in enumerate(
                            struct[s][j]):
                        s_ps = psum_s.tile([128, QCH], f32, tag="s_ps")
                        nc.tensor.matmul(
                            s_ps[:, :width],
                            lhsT=kt_sb[:, bass.ts(c, KT)],
                            rhs=qt_sb[:, j * QCH + st : (j + 1) * QCH],
                            start=True, stop=True,
                        )
                        if m_w > 0:
                            off, w = offsets[(s, j, c)]
                            m_sb = pm.tile([128, max_mw], f32)
                            nc.gpsimd.dma_start(
                                out=m_sb[:, :w],
                                in_=mask_d.ap()[s][:, off : off + w],
                            )
                            nc.vector.tensor_add(
                                s_ps[:, : m_w],
                                s_ps[:, : m_w],
                                m_sb[:, :w],
                            )
                        p_sb = pp.tile([128, QCH], f32r)
                        nc.scalar.activation(
                            p_sb[:, :width],
                            s_ps[:, :width],
                            mybir.ActivationFunctionType.Exp,
                            bias=bias_sb[:, c : c + 1],
                            scale=ALPHA,
                        )
                        nc.tensor.matmul(
                            ot_ps[:, st:],
                            lhsT=v_sb[:, bass.ts(c, DV)],
                            rhs=p_sb[:, :width],
                            start=(c == 0), stop=is_last,
                        )
                        nc.tensor.matmul(
                            den_ps[:, st:],
                            lhsT=ones_sb[:, 0:1],
                            rhs=p_sb[:, :width],
                            start=(c == 0), stop=is_last,
                        )
                        if idx == 2 and pending_epilogue is not None:
                            pending_epilogue()
                            pending_epilogue = None
                    # evacuate O^T and denominators; heavy epilogue is
                    # deferred into the next chunk's stream
                    otj_sb = pacc.tile([128, QCH], f32)
                    nc.vector.tensor_copy(otj_sb, ot_ps)
                    den_row = pacc.tile([1, QCH], f32)
                    nc.vector.tensor_copy(den_row, den_ps)
                    if pending_epilogue is not None:
                        pending_epilogue()
                    pending_epilogue = make_epilogue(s, j, otj_sb, den_row)
            pending_epilogue()
    nc.compile()
    return nc


def _prepare(queries, keys, values, valid_lens):
    """Host-side prep. Returns (key_sig, struct, offsets, total_w, in_maps,
    sortidx)."""
    queries = np.ascontiguousarray(np.asarray(queries, dtype=np.float32))
    keys = np.ascontiguousarray(np.asarray(keys, dtype=np.float32))
    values = np.ascontiguousarray(np.asarray(values, dtype=np.float32))
    vl = np.asarray(valid_lens, dtype=np.int64)

    # ---- host prep: per-batch sort by valid_len --------------------------
    sortidx = np.argsort(vl, axis=1, kind="stable")  # [B, Q]
    Ls = np.take_along_axis(vl, sortidx, axis=1)  # [B, Q] ascending

    # slot s of core n holds batch 2n + s
    Ls_by_slot = [Ls[s::BPC] for s in range(BPC)]  # each [8, Q]
    struct = _compute_structure(Ls_by_slot)
    Ls_by_core_slot = [[Ls[n * BPC + s] for s in range(BPC)]
                       for n in range(N_CORES)]
    offsets, total_w, masks = _build_masks(struct, Ls_by_core_slot)

    key_sig = (total_w, tuple(
        (s, j, c, st, width, m_lo, m_w, last)
        for s in range(BPC) for j in range(NJ)
        for (c, st, width, m_lo, m_w, last) in struct[s][j]
    ))

    # ---- per-core input maps --------------------------------------------
    biases = (D / 2.0 - 0.5 * (keys.astype(np.float64) ** 2).sum(-1)) * ALPHA
    biases = biases.astype(np.float32)  # [B, K]

    in_maps = []
    for n in range(N_CORES):
        qt = np.empty((BPC, D, Q), np.float32)
        kt = np.empty((BPC, D, K), np.float32)
        vp = np.empty((BPC, 128, NKT * DV), np.float32)
        bias_arr = np.empty((BPC, 128, NKT), np.float32)
        for s in range(BPC):
            b = n * BPC + s
            qt[s] = queries[b][sortidx[b]].T
            kt[s] = keys[b].T
            vp[s] = (values[b].reshape(NKT, 128, DV)
                     .transpose(1, 0, 2).reshape(128, NKT * DV))
            bias_arr[s] = biases[b].reshape(NKT, 128).T
        in_maps.append({
            "qt": qt, "kt": kt, "vp": vp, "bias": bias_arr,
            "masks": np.ascontiguousarray(masks[n]),
            "ones": np.concatenate(
                [np.ones((128, 1), np.float32),
                 np.zeros((128, 128), np.float32)], axis=1),
        })
    return key_sig, struct, offsets, total_w, in_maps, sortidx


def get_program(key_sig, struct, offsets, total_w):
    if key_sig not in _program_cache:
        _program_cache.clear()
        _program_cache[key_sig] = _build_program(struct, offsets, total_w)
    return _program_cache[key_sig]


def kernel(queries, keys, values, valid_lens):
    global LAST_EXEC_NS, LAST_WALL_S, LAST_RESULTS
    key_sig, struct, offsets, total_w, in_maps, sortidx = _prepare(
        queries, keys, values, valid_lens
    )
    nc = get_program(key_sig, struct, offsets, total_w)

    # ---- run on 8 cores --------------------------------------------------
    from concourse.bass_utils import run_bass_kernel_spmd

    trace = bool(int(os.environ.get("KBENCH_TRACE", "0")))
    kwargs = {}
    tdir = os.environ.get("KBENCH_TRACE_DIR")
    if trace and tdir:
        kwargs["tmpdir"] = tdir
    t0 = time.perf_counter()
    try:
        res = run_bass_kernel_spmd(
            nc, in_maps, core_ids=list(range(N_CORES)), trace=trace, **kwargs
        )
    except Exception:
        if not trace:
            raise
        import traceback
        traceback.print_exc()
        res = run_bass_kernel_spmd(
            nc, in_maps, core_ids=list(range(N_CORES)), trace=False
        )
    LAST_WALL_S = time.perf_counter() - t0
    LAST_EXEC_NS = res.exec_time_ns
    LAST_RESULTS = res

    # ---- gather + undo the sort -----------------------------------------
    out = np.empty((B, Q, DV), dtype=np.float32)
    for n in range(N_CORES):
        o = res.results[n]["out"]  # [BPC, 128, (Q//128)*DV]
        for s in range(BPC):
            b = n * BPC + s
            # device rows: q_sorted = 128*t + p  ->  [p, t, d]
            osort = (o[s].reshape(128, Q // 128, DV)
                     .transpose(1, 0, 2).reshape(Q, DV))
            out[b][sortidx[b]] = osort
    return out


# revision 25
# speedup vs baseline: 1.0786x; 1.0010x over previous
"""Trainium2 Bass kernel for masked dot-product-attention-with-distance.

Computes, for each batch b:
    raw    = Q @ K^T - 0.5*||k||^2          [Q, K]
    scaled = (raw + d/2) / sqrt(3d/2)
    masked softmax over k (k < valid_len[b, q]), then weights @ V.

Strategy:
  - Data-parallel over batch: 8 cores x 2 batches each.
  - Host: per batch, sort q rows by valid_len; pass Q^T and K^T layouts; fold
    the (d/2 - 0.5||k||^2)/sqrt(3d/2) term into a per-key bias applied on the
    ACT engine (exp(scale*S + bias)); precompute boundary 0/1 masks.
  - Device: S^T tiles [kpos=128, q<=512] via PE matmul (lhsT=K^T slice,
    rhs=Q^T slice); exp on ACT straight out of PSUM (scores are bounded, no
    max subtraction needed -- softmax is shift-invariant and exp stays in
    fp32 range); boundary masks multiplied in on DVE; O^T = V^T-stationary
    matmul accumulation over kpos tiles; denominator via ones-matmul;
    normalize with DVE reciprocal + PE transpose; DMA out.
  - Because q rows are sorted by valid_len, per (q-chunk, kpos-tile) ranges
    are trimmed at compile time (the program is specialized to the actual
    valid_lens): fully-masked regions are never computed and only boundary
    tiles pay masking cost.
"""

import math
import os
import time

import numpy as np

B, Q, K, D, DV = 16, 2048, 2048, 128, 128
N_CORES = 8
BPC = B // N_CORES  # batches per core (slots)
QCH = 512  # q chunk width (moving operand / PSUM bank)
NJ = Q // QCH  # 4
KT = 128  # kpos tile (contraction partition dim)
NKT = K // KT  # 16
ALPHA = float(1.0 / math.sqrt(3.0 * D / 2.0))

LAST_EXEC_NS = None
LAST_WALL_S = None
LAST_RESULTS = None

_program_cache = {}


def _compute_structure(Ls_by_slot):
    """Ls_by_slot[s] : [n_batches, Q] sorted valid_lens (ascending) for the
    batches mapped to slot s.  Returns per-slot compile-time structure:

    struct[s][j] = list of (c, st, width, m_lo, m_w, is_last) with
      st    : within-chunk q column where the matmul range starts (mult of 4)
      width : matmul free size = QCH - st
      m_lo  : mask window start (== st), m_w: mask window width (0 = no mask)
    """
    struct = []
    for s in range(BPC):
        Ls = Ls_by_slot[s]
        per_j = []
        for j in range(NJ):
            chunks = Ls[:, j * QCH : (j + 1) * QCH]  # [nb, QCH] sorted asc
            entries = []
            hot_cs = []
            for c in range(NKT):
                lo_key = c * KT  # L <= lo_key  -> tile c fully invalid
                hi_key = c * KT + KT - 1  # L <= hi_key -> needs masking
                qstart = int(
                    min(np.searchsorted(chunks[b], lo_key, side="right")
                        for b in range(chunks.shape[0]))
                )
                if qstart >= QCH:
                    break  # start is nondecreasing in c -> all later c skipped
                mend = int(
                    max(np.searchsorted(chunks[b], hi_key, side="right")
                        for b in range(chunks.shape[0]))
                )
                st = qstart & ~3  # align to 16B for PSUM-friendly APs
                m_hi = max(mend, qstart)
                m_w = m_hi - st if m_hi > st else 0
                hot_cs.append((c, st, QCH - st, st, m_w))
            for idx, (c, st, width, m_lo, m_w) in enumerate(hot_cs):
                entries.append((c, st, width, m_lo, m_w, idx == len(hot_cs) - 1))
            per_j.append(entries)
        struct.append(per_j)
    return struct


def _build_masks(struct, Ls_by_core_slot):
    """Lay out mask windows in a flat column blob (shared offsets across
    cores); returns (offsets dict {(s,j,c): (off,w)}, total_w, masks array
    [n_cores, BPC, 128, total_w] float32)."""
    offsets = {}
    off = 0
    for s in range(BPC):
        for j in range(NJ):
            for (c, st, width, m_lo, m_w, last) in struct[s][j]:
                if m_w > 0:
                    offsets[(s, j, c)] = (off, m_w)
                    off += m_w
    total_w = max(off, 4)
    # additive masks applied to raw scores pre-exp: 0.0 = valid,
    # -1e5 = invalid (exp underflows to exactly 0 after *ALPHA scaling)
    masks = np.zeros((N_CORES, BPC, 128, total_w), dtype=np.float32)
    kpos_col = np.arange(128, dtype=np.int64)[:, None]
    for (s, j, c), (o, w) in offsets.items():
        for n in range(N_CORES):
            Ls = Ls_by_core_slot[n][s]
            st = None
            for (cc, st_, width, m_lo, m_w, last) in struct[s][j]:
                if cc == c:
                    st = m_lo
                    break
            colL = Ls[j * QCH + st : j * QCH + st + w][None, :]  # [1, w]
            masks[n, s, :, o : o + w] = np.where(
                (kpos_col + c * KT) < colL, 0.0, -1e5
            ).astype(np.float32)
    return offsets, total_w, masks


def _build_program(struct, offsets, total_w):
    import concourse.bass as bass
    import concourse.bacc as bacc
    import concourse.mybir as mybir
    import concourse.tile as tile

    f32 = mybir.dt.float32
    f32r = mybir.dt.float32r  # single-pass PE matmul (full rate at N>=256)
    nc = bacc.Bacc("TRN2", target_bir_lowering=False, debug=False,
                   num_devices=N_CORES)

    qt_d = nc.dram_tensor("qt", [BPC, D, Q], f32r, kind="ExternalInput")
    kt_d = nc.dram_tensor("kt", [BPC, D, K], f32r, kind="ExternalInput")
    v_d = nc.dram_tensor("vp", [BPC, 128, NKT * DV], f32r, kind="ExternalInput")
    bias_d = nc.dram_tensor("bias", [BPC, 128, NKT], f32, kind="ExternalInput")
    mask_d = nc.dram_tensor("masks", [BPC, 128, total_w], f32,
                            kind="ExternalInput")
    ones_d = nc.dram_tensor("ones", [128, 129], f32r, kind="ExternalInput")
    out_d = nc.dram_tensor("out", [BPC, 128, (Q // 128) * DV], f32,
                           kind="ExternalOutput")

    max_mw = max([w for (_, w) in offsets.values()] + [4])

    with tile.TileContext(nc) as tc:
        with (
            tc.tile_pool(name="pin", bufs=2) as pin,
            tc.tile_pool(name="pconst", bufs=1) as pconst,
            tc.tile_pool(name="pp", bufs=5) as pp,
            tc.tile_pool(name="pm", bufs=4) as pm,
            tc.tile_pool(name="pacc", bufs=3) as pacc,
            tc.tile_pool(name="pout", bufs=3) as pout,
            tc.tile_pool(name="psum_s", bufs=3, space="PSUM") as psum_s,
            tc.tile_pool(name="psum_o", bufs=2, space="PSUM") as psum_o,
            tc.tile_pool(name="psum_d", bufs=2, space="PSUM") as psum_d,
            tc.tile_pool(name="psum_t", bufs=1, space="PSUM") as psum_t,
        ):
            ones_sb = pconst.tile([128, 129], f32r)
            nc.sync.dma_start(out=ones_sb, in_=ones_d.ap())
            ones_f = pconst.tile([128, 1], f32)
            nc.vector.memset(ones_f, 1.0)
            ident_sb = pconst.tile([128, 128], f32)
            from concourse.masks import make_identity
            make_identity(nc, ident_sb)
            # warm the ACT exp table set before real work arrives
            warm_in = pconst.tile([128, 1], f32)
            nc.vector.memset(warm_in, 0.0)
            warm_out = pconst.tile([128, 1], f32)
            nc.scalar.activation(warm_out, warm_in,
                                 mybir.ActivationFunctionType.Exp)

            def make_epilogue(s, j, otj_sb, den_row):
                def emit():
                    den_cols = psum_d.tile([128, QCH // 128], f32, tag="den",
                                           name="den_cols")
                    for t in range(QCH // 128):
                        nc.tensor.matmul(
                            den_cols[:, t : t + 1],
                            lhsT=den_row[0:1, bass.ts(t, 128)],
                            rhs=ones_f[0:1, 0:1],
                            start=True, stop=True,
                        )
                    recip_sb = pacc.tile([128, QCH // 128], f32,
                                         name="recip_sb")
                    nc.vector.reciprocal(recip_sb, den_cols)
                    o_ps = psum_t.tile([128, QCH], f32, name="o_ps")
                    for t in range(QCH // 128):
                        nc.tensor.transpose(
                            o_ps[:, bass.ts(t, 128)],
                            otj_sb[:, bass.ts(t, 128)], ident_sb,
                        )
                    out_sb = pout.tile([128, QCH // 128, DV], f32,
                                       name="out_sb")
                    for t in range(QCH // 128):
                        nc.vector.tensor_scalar_mul(
                            out_sb[:, t, :], in0=o_ps[:, bass.ts(t, 128)],
                            scalar1=recip_sb[:, t : t + 1],
                        )
                    nc.sync.dma_start(
                        out=out_d.ap()[s][:, j * QCH : (j + 1) * QCH]
                            .rearrange("p (t d) -> p t d", d=DV),
                        in_=out_sb,
                    )
                return emit

            pending_epilogue = None
            for s in range(BPC):
                bias_sb = pin.tile([128, NKT], f32)
                nc.sync.dma_start(out=bias_sb, in_=bias_d.ap()[s])
                kt_sb = pin.tile([128, K], f32r)
                qt_sb = pin.tile([128, Q], f32r)
                v_sb = pin.tile([128, NKT * DV], f32r)
                nc.sync.dma_start(out=kt_sb[:, 0:512],
                                  in_=kt_d.ap()[s][:, 0:512])
                nc.scalar.dma_start(out=qt_sb[:, 0:512],
                                    in_=qt_d.ap()[s][:, 0:512])
                nc.scalar.dma_start(out=v_sb[:, 0:512],
                                    in_=v_d.ap()[s][:, 0:512])
                nc.sync.dma_start(out=kt_sb[:, 512:K],
                                  in_=kt_d.ap()[s][:, 512:K])
                nc.scalar.dma_start(out=qt_sb[:, 512:Q],
                                    in_=qt_d.ap()[s][:, 512:Q])
                nc.scalar.dma_start(out=v_sb[:, 512:NKT * DV],
                                    in_=v_d.ap()[s][:, 512:NKT * DV])

                for j in range(NJ):
                    ot_ps = psum_o.tile([128, QCH], f32)
                    den_ps = psum_d.tile([1, QCH], f32, tag="den")
                    for idx, (c, st, width, m_lo, m_w, is_last) in enumerate(
                            struct[s][j]):
                        s_ps = psum_s.tile([128, QCH], f32, tag="s_ps")
                        nc.tensor.matmul(
                            s_ps[:, :width],
                            lhsT=kt_sb[:, bass.ts(c, KT)],
                            rhs=qt_sb[:, j * QCH + st : (j + 1) * QCH],
                            start=True, stop=True,
                        )
                        if m_w > 0:
                            off, w = offsets[(s, j, c)]
                            m_sb = pm.tile([128, max_mw], f32)
                            nc.gpsimd.dma_start(
                                out=m_sb[:, :w],
                                in_=mask_d.ap()[s][:, off : off + w],
                            )
                            nc.vector.tensor_add(
                                s_ps[:, : m_w],
                                s_ps[:, : m_w],
                                m_sb[:, :w],
                            )
                        p_sb = pp.tile([128, QCH], f32r)
                        nc.scalar.activation(
                            p_sb[:, :width],
                            s_ps[:, :width],
                            mybir.ActivationFunctionType.Exp,
                            bias=bias_sb[:, c : c + 1],
                            scale=ALPHA,
                        )
                        nc.tensor.matmul(
                            ot_ps[:, st:],
                            lhsT=v_sb[:, bass.ts(c, DV)],
                            rhs=p_sb[:, :width],
                            start=(c == 0), stop=is_last,
                        )
                        nc.tensor.matmul(
                            den_ps[:, st:],
                            lhsT=ones_sb[:, 0:1],
                            rhs=p_sb[:, :width],
                            start=(c == 0), stop=is_last,
                        )
                        if idx == 2 and pending_epilogue is not None:
                            pending_epilogue()
                            pending_epilogue = None
                    # evacuate O^T and denominators; heavy epilogue is
                    # deferred into the next chunk's stream
                    otj_sb = pacc.tile([128, QCH], f32)
                    nc.vector.tensor_copy(otj_sb, ot_ps)
                    den_row = pacc.tile([1, QCH], f32)
                    nc.vector.tensor_copy(den_row, den_ps)
                    if pending_epilogue is not None:
                        pending_epilogue()
                    pending_epilogue = make_epilogue(s, j, otj_sb, den_row)
            pending_epilogue()
    nc.compile()
    return nc


def _prepare(queries, keys, values, valid_lens):
    """Host-side prep. Returns (key_sig, struct, offsets, total_w, in_maps,
    sortidx)."""
    queries = np.ascontiguousarray(np.asarray(queries, dtype=np.float32))
    keys = np.ascontiguousarray(np.asarray(keys, dtype=np.float32))
    values = np.ascontiguousarray(np.asarray(values, dtype=np.float32))
    vl = np.asarray(valid_lens, dtype=np.int64)

    # ---- host prep: per-batch sort by valid_len --------------------------
    sortidx = np.argsort(vl, axis=1, kind="stable")  # [B, Q]
    Ls = np.take_along_axis(vl, sortidx, axis=1)  # [B, Q] ascending

    # slot s of core n holds batch 2n + s
    Ls_by_slot = [Ls[s::BPC] for s in range(BPC)]  # each [8, Q]
    struct = _compute_structure(Ls_by_slot)
    Ls_by_core_slot = [[Ls[n * BPC + s] for s in range(BPC)]
                       for n in range(N_CORES)]
    offsets, total_w, masks = _build_masks(struct, Ls_by_core_slot)

    key_sig = (total_w, tuple(
        (s, j, c, st, width, m_lo, m_w, last)
        for s in range(BPC) for j in range(NJ)
        for (c, st, width, m_lo, m_w, last) in struct[s][j]
    ))

    # ---- per-core input maps --------------------------------------------
    biases = (D / 2.0 - 0.5 * (keys.astype(np.float64) ** 2).sum(-1)) * ALPHA
    biases = biases.astype(np.float32)  # [B, K]

    in_maps = []
    for n in range(N_CORES):
        qt = np.empty((BPC, D, Q), np.float32)
        kt = np.empty((BPC, D, K), np.float32)
        vp = np.empty((BPC, 128, NKT * DV), np.float32)
        bias_arr = np.empty((BPC, 128, NKT), np.float32)
        for s in range(BPC):
            b = n * BPC + s
            qt[s] = queries[b][sortidx[b]].T
            kt[s] = keys[b].T
            vp[s] = (values[b].reshape(NKT, 128, DV)
                     .transpose(1, 0, 2).reshape(128, NKT * DV))
            bias_arr[s] = biases[b].reshape(NKT, 128).T
        in_maps.append({
            "qt": qt, "kt": kt, "vp": vp, "bias": bias_arr,
            "masks": np.ascontiguousarray(masks[n]),
            "ones": np.concatenate(
                [np.ones((128, 1), np.float32),
                 np.zeros((128, 128), np.float32)], axis=1),
        })
    return key_sig, struct, offsets, total_w, in_maps, sortidx


def get_program(key_sig, struct, offsets, total_w):
    if key_sig not in _program_cache:
        _program_cache.clear()
        _program_cache[key_sig] = _build_program(struct, offsets, total_w)
    return _program_cache[key_sig]


def kernel(queries, keys, values, valid_lens):
    global LAST_EXEC_NS, LAST_WALL_S, LAST_RESULTS
    key_sig, struct, offsets, total_w, in_maps, sortidx = _prepare(
        queries, keys, values, valid_lens
    )
    nc = get_program(key_sig, struct, offsets, total_w)

    # ---- run on 8 cores --------------------------------------------------
    from concourse.bass_utils import run_bass_kernel_spmd

    trace = bool(int(os.environ.get("KBENCH_TRACE", "0")))
    kwargs = {}
    tdir = os.environ.get("KBENCH_TRACE_DIR")
    if trace and tdir:
        kwargs["tmpdir"] = tdir
    t0 = time.perf_counter()
    try:
        res = run_bass_kernel_spmd(
            nc, in_maps, core_ids=list(range(N_CORES)), trace=trace, **kwargs
        )
    except Exception:
        if not trace:
            raise
        import traceback
        traceback.print_exc()
        res = run_bass_kernel_spmd(
            nc, in_maps, core_ids=list(range(N_CORES)), trace=False
        )
    LAST_WALL_S = time.perf_counter() - t0
    LAST_EXEC_NS = res.exec_time_ns
    LAST_RESULTS = res

    # ---- gather + undo the sort -----------------------------------------
    out = np.empty((B, Q, DV), dtype=np.float32)
    for n in range(N_CORES):
        o = res.results[n]["out"]  # [BPC, 128, (Q//128)*DV]
        for s in range(BPC):
            b = n * BPC + s
            # device rows: q_sorted = 128*t + p  ->  [p, t, d]
            osort = (o[s].reshape(128, Q // 128, DV)
                     .transpose(1, 0, 2).reshape(Q, DV))
            out[b][sortidx[b]] = osort
    return out


# revision 26
# speedup vs baseline: 1.2110x; 1.1228x over previous
"""Trainium2 Bass kernel for masked dot-product-attention-with-distance.

Computes, for each batch b:
    raw    = Q @ K^T - 0.5*||k||^2          [Q, K]
    scaled = (raw + d/2) / sqrt(3d/2)
    masked softmax over k (k < valid_len[b, q]), then weights @ V.

Strategy:
  - Data-parallel over batch: 8 cores x 2 batches each.
  - Host: per batch, sort q rows by valid_len; pass Q^T and K^T layouts; fold
    the (d/2 - 0.5||k||^2)/sqrt(3d/2) term into a per-key bias applied on the
    ACT engine (exp(scale*S + bias)); precompute boundary 0/1 masks.
  - Device: S^T tiles [kpos=128, q<=512] via PE matmul (lhsT=K^T slice,
    rhs=Q^T slice); exp on ACT straight out of PSUM (scores are bounded, no
    max subtraction needed -- softmax is shift-invariant and exp stays in
    fp32 range); boundary masks multiplied in on DVE; O^T = V^T-stationary
    matmul accumulation over kpos tiles; denominator via ones-matmul;
    normalize with DVE reciprocal + PE transpose; DMA out.
  - Because q rows are sorted by valid_len, per (q-chunk, kpos-tile) ranges
    are trimmed at compile time (the program is specialized to the actual
    valid_lens): fully-masked regions are never computed and only boundary
    tiles pay masking cost.
"""

import math
import os
import time

import numpy as np

B, Q, K, D, DV = 16, 2048, 2048, 128, 128
N_CORES = 8
BPC = B // N_CORES  # batches per core (slots)
QCH = 512  # q chunk width (moving operand / PSUM bank)
NJ = Q // QCH  # 4
KT = 128  # kpos tile (contraction partition dim)
NKT = K // KT  # 16
ALPHA = float(1.0 / math.sqrt(3.0 * D / 2.0))

LAST_EXEC_NS = None
LAST_WALL_S = None
LAST_RESULTS = None

_program_cache = {}


def _compute_structure(Ls_by_slot):
    """Ls_by_slot[s] : [n_batches, Q] sorted valid_lens (ascending) for the
    batches mapped to slot s.  Returns per-slot compile-time structure:

    struct[s][j] = list of (c, st, width, m_lo, m_w, is_last) with
      st    : within-chunk q column where the matmul range starts (mult of 4)
      width : matmul free size = QCH - st
      m_lo  : mask window start (== st), m_w: mask window width (0 = no mask)
    """
    struct = []
    for s in range(BPC):
        Ls = Ls_by_slot[s]
        per_j = []
        for j in range(NJ):
            chunks = Ls[:, j * QCH : (j + 1) * QCH]  # [nb, QCH] sorted asc
            entries = []
            hot_cs = []
            for c in range(NKT):
                lo_key = c * KT  # L <= lo_key  -> tile c fully invalid
                hi_key = c * KT + KT - 1  # L <= hi_key -> needs masking
                qstart = int(
                    min(np.searchsorted(chunks[b], lo_key, side="right")
                        for b in range(chunks.shape[0]))
                )
                if qstart >= QCH:
                    break  # start is nondecreasing in c -> all later c skipped
                mend = int(
                    max(np.searchsorted(chunks[b], hi_key, side="right")
                        for b in range(chunks.shape[0]))
                )
                st = qstart & ~3  # align to 16B for PSUM-friendly APs
                m_hi = max(mend, qstart)
                m_w = m_hi - st if m_hi > st else 0
                hot_cs.append((c, st, QCH - st, st, m_w))
            for idx, (c, st, width, m_lo, m_w) in enumerate(hot_cs):
                entries.append((c, st, width, m_lo, m_w, idx == len(hot_cs) - 1))
            per_j.append(entries)
        struct.append(per_j)
    return struct


def _build_masks(struct, Ls_by_core_slot):
    """Lay out mask windows in a flat column blob (shared offsets across
    cores); returns (offsets dict {(s,j,c): (off,w)}, total_w, masks array
    [n_cores, BPC, 128, total_w] float32)."""
    offsets = {}
    off = 0
    for s in range(BPC):
        for j in range(NJ):
            for (c, st, width, m_lo, m_w, last) in struct[s][j]:
                if m_w > 0:
                    offsets[(s, j, c)] = (off, m_w)
                    off += m_w
    total_w = max(off, 4)
    # additive masks applied to raw scores pre-exp: 0.0 = valid,
    # -1e5 = invalid (exp underflows to exactly 0 after *ALPHA scaling)
    masks = np.zeros((N_CORES, BPC, 128, total_w), dtype=np.float32)
    kpos_col = np.arange(128, dtype=np.int64)[:, None]
    for (s, j, c), (o, w) in offsets.items():
        for n in range(N_CORES):
            Ls = Ls_by_core_slot[n][s]
            st = None
            for (cc, st_, width, m_lo, m_w, last) in struct[s][j]:
                if cc == c:
                    st = m_lo
                    break
            colL = Ls[j * QCH + st : j * QCH + st + w][None, :]  # [1, w]
            masks[n, s, :, o : o + w] = np.where(
                (kpos_col + c * KT) < colL, 0.0, -1e5
            ).astype(np.float32)
    return offsets, total_w, masks


def _build_program(struct, offsets, total_w):
    import concourse.bass as bass
    import concourse.bacc as bacc
    import concourse.mybir as mybir
    import concourse.tile as tile

    f32 = mybir.dt.float32
    f32r = mybir.dt.float32r  # single-pass PE matmul (full rate at N>=256)
    nc = bacc.Bacc("TRN2", target_bir_lowering=False, debug=False,
                   num_devices=N_CORES)

    qt_d = nc.dram_tensor("qt", [BPC, D, Q], f32r, kind="ExternalInput")
    kt_d = nc.dram_tensor("kt", [BPC, D, K], f32r, kind="ExternalInput")
    v_d = nc.dram_tensor("vp", [BPC, 128, NKT * DV], f32r, kind="ExternalInput")
    bias_d = nc.dram_tensor("bias", [BPC, 128, NKT], f32, kind="ExternalInput")
    mask_d = nc.dram_tensor("masks", [BPC, 128, total_w], f32,
                            kind="ExternalInput")
    ones_d = nc.dram_tensor("ones", [128, 129], f32r, kind="ExternalInput")
    out_d = nc.dram_tensor("out", [BPC, 128, (Q // 128) * DV], f32,
                           kind="ExternalOutput")

    max_mw = max([w for (_, w) in offsets.values()] + [4])

    with tile.TileContext(nc) as tc:
        with (
            tc.tile_pool(name="pin", bufs=2) as pin,
            tc.tile_pool(name="pconst", bufs=1) as pconst,
            tc.tile_pool(name="pp", bufs=5) as pp,
            tc.tile_pool(name="pm", bufs=6) as pm,
            tc.tile_pool(name="pacc", bufs=3) as pacc,
            tc.tile_pool(name="pout", bufs=3) as pout,
            tc.tile_pool(name="psum_s", bufs=4, space="PSUM") as psum_s,
            tc.tile_pool(name="psum_o", bufs=2, space="PSUM") as psum_o,
            tc.tile_pool(name="psum_d", bufs=2, space="PSUM") as psum_d,
        ):
            ones_sb = pconst.tile([128, 129], f32r)
            nc.scalar.dma_start(out=ones_sb, in_=ones_d.ap())
            ones_f = pconst.tile([128, 1], f32)
            nc.vector.memset(ones_f, 1.0)
            ident_sb = pconst.tile([128, 128], f32)
            from concourse.masks import make_identity
            make_identity(nc, ident_sb)
            # warm the ACT exp table set before real work arrives
            warm_in = pconst.tile([128, 1], f32)
            nc.vector.memset(warm_in, 0.0)
            warm_out = pconst.tile([128, 1], f32)
            nc.scalar.activation(warm_out, warm_in,
                                 mybir.ActivationFunctionType.Exp)

            def make_epilogue(s, j, otj_sb, den_row):
                def emit():
                    den_cols = psum_d.tile([128, QCH // 128], f32, tag="den",
                                           name="den_cols")
                    for t in range(QCH // 128):
                        nc.tensor.matmul(
                            den_cols[:, t : t + 1],
                            lhsT=den_row[0:1, bass.ts(t, 128)],
                            rhs=ones_f[0:1, 0:1],
                            start=True, stop=True,
                        )
                    recip_sb = pacc.tile([128, QCH // 128], f32,
                                         name="recip_sb")
                    nc.vector.reciprocal(recip_sb, den_cols)
                    o_ps = psum_o.tile([128, QCH], f32, tag="ot_ps",
                                        name="o_ps")
                    for t in range(QCH // 128):
                        nc.tensor.transpose(
                            o_ps[:, bass.ts(t, 128)],
                            otj_sb[:, bass.ts(t, 128)], ident_sb,
                        )
                    out_sb = pout.tile([128, QCH // 128, DV], f32,
                                       name="out_sb")
                    for t in range(QCH // 128):
                        nc.vector.tensor_scalar_mul(
                            out_sb[:, t, :], in0=o_ps[:, bass.ts(t, 128)],
                            scalar1=recip_sb[:, t : t + 1],
                        )
                    nc.sync.dma_start(
                        out=out_d.ap()[s][:, j * QCH : (j + 1) * QCH]
                            .rearrange("p (t d) -> p t d", d=DV),
                        in_=out_sb,
                    )
                return emit

            pending_epilogue = None
            for s in range(BPC):
                bias_sb = pin.tile([128, NKT], f32)
                nc.scalar.dma_start(out=bias_sb, in_=bias_d.ap()[s])
                kt_sb = pin.tile([128, K], f32r)
                qt_sb = pin.tile([128, Q], f32r)
                v_sb = pin.tile([128, NKT * DV], f32r)
                nc.sync.dma_start(out=kt_sb[:, 0:512],
                                  in_=kt_d.ap()[s][:, 0:512])
                nc.scalar.dma_start(out=qt_sb[:, 0:512],
                                    in_=qt_d.ap()[s][:, 0:512])
                nc.scalar.dma_start(out=v_sb[:, 0:512],
                                    in_=v_d.ap()[s][:, 0:512])
                nc.sync.dma_start(out=kt_sb[:, 512:K],
                                  in_=kt_d.ap()[s][:, 512:K])
                nc.scalar.dma_start(out=qt_sb[:, 512:Q],
                                    in_=qt_d.ap()[s][:, 512:Q])
                nc.scalar.dma_start(out=v_sb[:, 512:NKT * DV],
                                    in_=v_d.ap()[s][:, 512:NKT * DV])

                for j in range(NJ):
                    ot_ps = psum_o.tile([128, QCH], f32)
                    den_ps = psum_d.tile([1, QCH], f32, tag="den")
                    for idx, (c, st, width, m_lo, m_w, is_last) in enumerate(
                            struct[s][j]):
                        s_ps = psum_s.tile([128, QCH], f32, tag="s_ps")
                        nc.tensor.matmul(
                            s_ps[:, :width],
                            lhsT=kt_sb[:, bass.ts(c, KT)],
                            rhs=qt_sb[:, j * QCH + st : (j + 1) * QCH],
                            start=True, stop=True,
                        )
                        if m_w > 0:
                            off, w = offsets[(s, j, c)]
                            m_sb = pm.tile([128, max_mw], f32)
                            nc.gpsimd.dma_start(
                                out=m_sb[:, :w],
                                in_=mask_d.ap()[s][:, off : off + w],
                            )
                            nc.vector.tensor_add(
                                s_ps[:, : m_w],
                                s_ps[:, : m_w],
                                m_sb[:, :w],
                            )
                        p_sb = pp.tile([128, QCH], f32r)
                        nc.scalar.activation(
                            p_sb[:, :width],
                            s_ps[:, :width],
                            mybir.ActivationFunctionType.Exp,
                            bias=bias_sb[:, c : c + 1],
                            scale=ALPHA,
                        )
                        nc.tensor.matmul(
                            ot_ps[:, st:],
                            lhsT=v_sb[:, bass.ts(c, DV)],
                            rhs=p_sb[:, :width],
                            start=(c == 0), stop=is_last,
                        )
                        nc.tensor.matmul(
                            den_ps[:, st:],
                            lhsT=ones_sb[:, 0:1],
                            rhs=p_sb[:, :width],
                            start=(c == 0), stop=is_last,
                        )
                        if idx == 2 and pending_epilogue is not None:
                            pending_epilogue()
                            pending_epilogue = None
                    # evacuate O^T and denominators; heavy epilogue is
                    # deferred into the next chunk's stream
                    otj_sb = pacc.tile([128, QCH], f32)
                    nc.vector.tensor_copy(otj_sb, ot_ps)
                    den_row = pacc.tile([1, QCH], f32)
                    nc.vector.tensor_copy(den_row, den_ps)
                    if pending_epilogue is not None:
                        pending_epilogue()
                    pending_epilogue = make_epilogue(s, j, otj_sb, den_row)
            pending_epilogue()
    nc.compile()
    return nc


def _prepare(queries, keys, values, valid_lens):
    """Host-side prep. Returns (key_sig, struct, offsets, total_w, in_maps,
    sortidx)."""
    queries = np.ascontiguousarray(np.asarray(queries, dtype=np.float32))
    keys = np.ascontiguousarray(np.asarray(keys, dtype=np.float32))
    values = np.ascontiguousarray(np.asarray(values, dtype=np.float32))
    vl = np.asarray(valid_lens, dtype=np.int64)

    # ---- host prep: per-batch sort by valid_len --------------------------
    sortidx = np.argsort(vl, axis=1, kind="stable")  # [B, Q]
    Ls = np.take_along_axis(vl, sortidx, axis=1)  # [B, Q] ascending

    # slot s of core n holds batch 2n + s
    Ls_by_slot = [Ls[s::BPC] for s in range(BPC)]  # each [8, Q]
    struct = _compute_structure(Ls_by_slot)
    Ls_by_core_slot = [[Ls[n * BPC + s] for s in range(BPC)]
                       for n in range(N_CORES)]
    offsets, total_w, masks = _build_masks(struct, Ls_by_core_slot)

    key_sig = (total_w, tuple(
        (s, j, c, st, width, m_lo, m_w, last)
        for s in range(BPC) for j in range(NJ)
        for (c, st, width, m_lo, m_w, last) in struct[s][j]
    ))

    # ---- per-core input maps --------------------------------------------
    biases = (D / 2.0 - 0.5 * (keys.astype(np.float64) ** 2).sum(-1)) * ALPHA
    biases = biases.astype(np.float32)  # [B, K]

    in_maps = []
    for n in range(N_CORES):
        qt = np.empty((BPC, D, Q), np.float32)
        kt = np.empty((BPC, D, K), np.float32)
        vp = np.empty((BPC, 128, NKT * DV), np.float32)
        bias_arr = np.empty((BPC, 128, NKT), np.float32)
        for s in range(BPC):
            b = n * BPC + s
            qt[s] = queries[b][sortidx[b]].T
            kt[s] = keys[b].T
            vp[s] = (values[b].reshape(NKT, 128, DV)
                     .transpose(1, 0, 2).reshape(128, NKT * DV))
            bias_arr[s] = biases[b].reshape(NKT, 128).T
        in_maps.append({
            "qt": qt, "kt": kt, "vp": vp, "bias": bias_arr,
            "masks": np.ascontiguousarray(masks[n]),
            "ones": np.concatenate(
                [np.ones((128, 1), np.float32),
                 np.zeros((128, 128), np.float32)], axis=1),
        })
    return key_sig, struct, offsets, total_w, in_maps, sortidx


def get_program(key_sig, struct, offsets, total_w):
    if key_sig not in _program_cache:
        _program_cache.clear()
        _program_cache[key_sig] = _build_program(struct, offsets, total_w)
    return _program_cache[key_sig]


def kernel(queries, keys, values, valid_lens):
    global LAST_EXEC_NS, LAST_WALL_S, LAST_RESULTS
    key_sig, struct, offsets, total_w, in_maps, sortidx = _prepare(
        queries, keys, values, valid_lens
    )
    nc = get_program(key_sig, struct, offsets, total_w)

    # ---- run on 8 cores --------------------------------------------------
    from concourse.bass_utils import run_bass_kernel_spmd

    trace = bool(int(os.environ.get("KBENCH_TRACE", "0")))
    kwargs = {}
    tdir = os.environ.get("KBENCH_TRACE_DIR")
    if trace and tdir:
        kwargs["tmpdir"] = tdir
    t0 = time.perf_counter()
    try:
        res = run_bass_kernel_spmd(
            nc, in_maps, core_ids=list(range(N_CORES)), trace=trace, **kwargs
        )
    except Exception:
        if not trace:
            raise
        import traceback
        traceback.print_exc()
        res = run_bass_kernel_spmd(
            nc, in_maps, core_ids=list(range(N_CORES)), trace=False
        )
    LAST_WALL_S = time.perf_counter() - t0
    LAST_EXEC_NS = res.exec_time_ns
    LAST_RESULTS = res

    # ---- gather + undo the sort -----------------------------------------
    out = np.empty((B, Q, DV), dtype=np.float32)
    for n in range(N_CORES):
        o = res.results[n]["out"]  # [BPC, 128, (Q//128)*DV]
        for s in range(BPC):
            b = n * BPC + s
            # device rows: q_sorted = 128*t + p  ->  [p, t, d]
            osort = (o[s].reshape(128, Q // 128, DV)
                     .transpose(1, 0, 2).reshape(Q, DV))
            out[b][sortidx[b]] = osort
    return out


# revision 27
# speedup vs baseline: 1.2170x; 1.0050x over previous
"""Trainium2 Bass kernel for masked dot-product-attention-with-distance.

Computes, for each batch b:
    raw    = Q @ K^T - 0.5*||k||^2          [Q, K]
    scaled = (raw + d/2) / sqrt(3d/2)
    masked softmax over k (k < valid_len[b, q]), then weights @ V.

Strategy:
  - Data-parallel over batch: 8 cores x 2 batches each.
  - Host: per batch, sort q rows by valid_len; pass Q^T and K^T layouts; fold
    the (d/2 - 0.5||k||^2)/sqrt(3d/2) term into a per-key bias applied on the
    ACT engine (exp(scale*S + bias)); precompute boundary 0/1 masks.
  - Device: S^T tiles [kpos=128, q<=512] via PE matmul (lhsT=K^T slice,
    rhs=Q^T slice); exp on ACT straight out of PSUM (scores are bounded, no
    max subtraction needed -- softmax is shift-invariant and exp stays in
    fp32 range); boundary masks multiplied in on DVE; O^T = V^T-stationary
    matmul accumulation over kpos tiles; denominator via ones-matmul;
    normalize with DVE reciprocal + PE transpose; DMA out.
  - Because q rows are sorted by valid_len, per (q-chunk, kpos-tile) ranges
    are trimmed at compile time (the program is specialized to the actual
    valid_lens): fully-masked regions are never computed and only boundary
    tiles pay masking cost.
"""

import math
import os
import time

import numpy as np

B, Q, K, D, DV = 16, 2048, 2048, 128, 128
N_CORES = 8
BPC = B // N_CORES  # batches per core (slots)
QCH = 512  # q chunk width (moving operand / PSUM bank)
NJ = Q // QCH  # 4
KT = 128  # kpos tile (contraction partition dim)
NKT = K // KT  # 16
ALPHA = float(1.0 / math.sqrt(3.0 * D / 2.0))

LAST_EXEC_NS = None
LAST_WALL_S = None
LAST_RESULTS = None

_program_cache = {}


def _compute_structure(Ls_by_slot):
    """Ls_by_slot[s] : [n_batches, Q] sorted valid_lens (ascending) for the
    batches mapped to slot s.  Returns per-slot compile-time structure:

    struct[s][j] = list of (c, st, width, m_lo, m_w, is_last) with
      st    : within-chunk q column where the matmul range starts (mult of 4)
      width : matmul free size = QCH - st
      m_lo  : mask window start (== st), m_w: mask window width (0 = no mask)
    """
    struct = []
    for s in range(BPC):
        Ls = Ls_by_slot[s]
        per_j = []
        for j in range(NJ):
            chunks = Ls[:, j * QCH : (j + 1) * QCH]  # [nb, QCH] sorted asc
            entries = []
            hot_cs = []
            for c in range(NKT):
                lo_key = c * KT  # L <= lo_key  -> tile c fully invalid
                hi_key = c * KT + KT - 1  # L <= hi_key -> needs masking
                qstart = int(
                    min(np.searchsorted(chunks[b], lo_key, side="right")
                        for b in range(chunks.shape[0]))
                )
                if qstart >= QCH:
                    break  # start is nondecreasing in c -> all later c skipped
                mend = int(
                    max(np.searchsorted(chunks[b], hi_key, side="right")
                        for b in range(chunks.shape[0]))
                )
                st = qstart & ~3  # align to 16B for PSUM-friendly APs
                m_hi = max(mend, qstart)
                m_w = m_hi - st if m_hi > st else 0
                hot_cs.append((c, st, QCH - st, st, m_w))
            for idx, (c, st, width, m_lo, m_w) in enumerate(hot_cs):
                entries.append((c, st, width, m_lo, m_w, idx == len(hot_cs) - 1))
            per_j.append(entries)
        struct.append(per_j)
    return struct


def _build_masks(struct, Ls_by_core_slot):
    """Lay out mask windows in a flat column blob (shared offsets across
    cores); returns (offsets dict {(s,j,c): (off,w)}, total_w, masks array
    [n_cores, BPC, 128, total_w] float32)."""
    offsets = {}
    off = 0
    for s in range(BPC):
        for j in range(NJ):
            for (c, st, width, m_lo, m_w, last) in struct[s][j]:
                if m_w > 0:
                    offsets[(s, j, c)] = (off, m_w)
                    off += m_w
    total_w = max(off, 4)
    # additive masks applied to raw scores pre-exp: 0.0 = valid,
    # -1e5 = invalid (exp underflows to exactly 0 after *ALPHA scaling)
    masks = np.zeros((N_CORES, BPC, 128, total_w), dtype=np.float32)
    kpos_col = np.arange(128, dtype=np.int64)[:, None]
    for (s, j, c), (o, w) in offsets.items():
        for n in range(N_CORES):
            Ls = Ls_by_core_slot[n][s]
            st = None
            for (cc, st_, width, m_lo, m_w, last) in struct[s][j]:
                if cc == c:
                    st = m_lo
                    break
            colL = Ls[j * QCH + st : j * QCH + st + w][None, :]  # [1, w]
            masks[n, s, :, o : o + w] = np.where(
                (kpos_col + c * KT) < colL, 0.0, -1e5
            ).astype(np.float32)
    return offsets, total_w, masks


def _build_program(struct, offsets, total_w):
    import concourse.bass as bass
    import concourse.bacc as bacc
    import concourse.mybir as mybir
    import concourse.tile as tile

    f32 = mybir.dt.float32
    f32r = mybir.dt.float32r  # single-pass PE matmul (full rate at N>=256)
    nc = bacc.Bacc("TRN2", target_bir_lowering=False, debug=False,
                   num_devices=N_CORES)

    qt_d = nc.dram_tensor("qt", [BPC, D, Q], f32r, kind="ExternalInput")
    kt_d = nc.dram_tensor("kt", [BPC, D, K], f32r, kind="ExternalInput")
    v_d = nc.dram_tensor("vp", [BPC, 128, NKT * DV], f32r, kind="ExternalInput")
    bias_d = nc.dram_tensor("bias", [BPC, 128, NKT], f32, kind="ExternalInput")
    mask_d = nc.dram_tensor("masks", [BPC, 128, total_w], f32,
                            kind="ExternalInput")
    ones_d = nc.dram_tensor("ones", [128, 129], f32r, kind="ExternalInput")
    out_d = nc.dram_tensor("out", [BPC, 128, (Q // 128) * DV], f32,
                           kind="ExternalOutput")

    max_mw = max([w for (_, w) in offsets.values()] + [4])

    with tile.TileContext(nc) as tc:
        with (
            tc.tile_pool(name="pin", bufs=2) as pin,
            tc.tile_pool(name="pconst", bufs=1) as pconst,
            tc.tile_pool(name="pp", bufs=5) as pp,
            tc.tile_pool(name="pm", bufs=6) as pm,
            tc.tile_pool(name="pacc", bufs=3) as pacc,
            tc.tile_pool(name="pout", bufs=3) as pout,
            tc.tile_pool(name="psum_s", bufs=4, space="PSUM") as psum_s,
            tc.tile_pool(name="psum_o", bufs=2, space="PSUM") as psum_o,
            tc.tile_pool(name="psum_d", bufs=2, space="PSUM") as psum_d,
        ):
            ones_sb = pconst.tile([128, 129], f32r)
            nc.scalar.dma_start(out=ones_sb, in_=ones_d.ap())
            ones_f = pconst.tile([128, 1], f32)
            nc.vector.memset(ones_f, 1.0)
            ident_sb = pconst.tile([128, 128], f32)
            from concourse.masks import make_identity
            make_identity(nc, ident_sb)
            # warm the ACT exp table set before real work arrives
            warm_in = pconst.tile([128, 1], f32)
            nc.vector.memset(warm_in, 0.0)
            warm_out = pconst.tile([128, 1], f32)
            nc.scalar.activation(warm_out, warm_in,
                                 mybir.ActivationFunctionType.Exp)

            def make_epilogue(s, j, otj_sb, den_row):
                def emit():
                    den_cols = psum_d.tile([128, QCH // 128], f32, tag="den",
                                           name="den_cols")
                    for t in range(QCH // 128):
                        nc.tensor.matmul(
                            den_cols[:, t : t + 1],
                            lhsT=den_row[0:1, bass.ts(t, 128)],
                            rhs=ones_f[0:1, 0:1],
                            start=True, stop=True,
                        )
                    recip_sb = pacc.tile([128, QCH // 128], f32,
                                         name="recip_sb")
                    nc.vector.reciprocal(recip_sb, den_cols)
                    o_ps = psum_o.tile([128, QCH], f32, tag="ot_ps",
                                        name="o_ps")
                    for t in range(QCH // 128):
                        nc.tensor.transpose(
                            o_ps[:, bass.ts(t, 128)],
                            otj_sb[:, bass.ts(t, 128)], ident_sb,
                        )
                    out_sb = pout.tile([128, QCH // 128, DV], f32,
                                       name="out_sb")
                    for t in range(QCH // 128):
                        nc.vector.tensor_scalar_mul(
                            out_sb[:, t, :], in0=o_ps[:, bass.ts(t, 128)],
                            scalar1=recip_sb[:, t : t + 1],
                        )
                    nc.sync.dma_start(
                        out=out_d.ap()[s][:, j * QCH : (j + 1) * QCH]
                            .rearrange("p (t d) -> p t d", d=DV),
                        in_=out_sb,
                    )
                return emit

            pending_epilogue = None
            for s in range(BPC):
                bias_sb = pin.tile([128, NKT], f32)
                nc.scalar.dma_start(out=bias_sb, in_=bias_d.ap()[s])
                kt_sb = pin.tile([128, K], f32r)
                qt_sb = pin.tile([128, Q], f32r)
                v_sb = pin.tile([128, NKT * DV], f32r)
                nc.sync.dma_start(out=kt_sb[:, 0:512],
                                  in_=kt_d.ap()[s][:, 0:512])
                nc.scalar.dma_start(out=qt_sb[:, 0:512],
                                    in_=qt_d.ap()[s][:, 0:512])
                nc.scalar.dma_start(out=v_sb[:, 0:512],
                                    in_=v_d.ap()[s][:, 0:512])
                if s == 0:
                    nc.scalar.dma_start(out=ones_sb, in_=ones_d.ap())
                nc.sync.dma_start(out=kt_sb[:, 512:K],
                                  in_=kt_d.ap()[s][:, 512:K])
                nc.scalar.dma_start(out=qt_sb[:, 512:Q],
                                    in_=qt_d.ap()[s][:, 512:Q])
                nc.scalar.dma_start(out=v_sb[:, 512:NKT * DV],
                                    in_=v_d.ap()[s][:, 512:NKT * DV])

                for j in range(NJ):
                    ot_ps = psum_o.tile([128, QCH], f32)
                    den_ps = psum_d.tile([1, QCH], f32, tag="den")
                    for idx, (c, st, width, m_lo, m_w, is_last) in enumerate(
                            struct[s][j]):
                        s_ps = psum_s.tile([128, QCH], f32, tag="s_ps")
                        nc.tensor.matmul(
                            s_ps[:, :width],
                            lhsT=kt_sb[:, bass.ts(c, KT)],
                            rhs=qt_sb[:, j * QCH + st : (j + 1) * QCH],
                            start=True, stop=True,
                        )
                        if m_w > 0:
                            off, w = offsets[(s, j, c)]
                            m_sb = pm.tile([128, max_mw], f32)
                            nc.gpsimd.dma_start(
                                out=m_sb[:, :w],
                                in_=mask_d.ap()[s][:, off : off + w],
                            )
                            nc.vector.tensor_add(
                                s_ps[:, : m_w],
                                s_ps[:, : m_w],
                                m_sb[:, :w],
                            )
                        p_sb = pp.tile([128, QCH], f32r)
                        nc.scalar.activation(
                            p_sb[:, :width],
                            s_ps[:, :width],
                            mybir.ActivationFunctionType.Exp,
                            bias=bias_sb[:, c : c + 1],
                            scale=ALPHA,
                        )
                        nc.tensor.matmul(
                            ot_ps[:, st:],
                            lhsT=v_sb[:, bass.ts(c, DV)],
                            rhs=p_sb[:, :width],
                            start=(c == 0), stop=is_last,
                        )
                        nc.tensor.matmul(
                            den_ps[:, st:],
                            lhsT=ones_sb[:, 0:1],
                            rhs=p_sb[:, :width],
                            start=(c == 0), stop=is_last,
                        )
                        if idx == 2 and pending_epilogue is not None:
                            pending_epilogue()
                            pending_epilogue = None
                    # evacuate O^T and denominators; heavy epilogue is
                    # deferred into the next chunk's stream
                    otj_sb = pacc.tile([128, QCH], f32)
                    nc.vector.tensor_copy(otj_sb, ot_ps)
                    den_row = pacc.tile([1, QCH], f32)
                    nc.vector.tensor_copy(den_row, den_ps)
                    if pending_epilogue is not None:
                        pending_epilogue()
                    pending_epilogue = make_epilogue(s, j, otj_sb, den_row)
            pending_epilogue()
    nc.compile()
    return nc


def _prepare(queries, keys, values, valid_lens):
    """Host-side prep. Returns (key_sig, struct, offsets, total_w, in_maps,
    sortidx)."""
    queries = np.ascontiguousarray(np.asarray(queries, dtype=np.float32))
    keys = np.ascontiguousarray(np.asarray(keys, dtype=np.float32))
    values = np.ascontiguousarray(np.asarray(values, dtype=np.float32))
    vl = np.asarray(valid_lens, dtype=np.int64)

    # ---- host prep: per-batch sort by valid_len --------------------------
    sortidx = np.argsort(vl, axis=1, kind="stable")  # [B, Q]
    Ls = np.take_along_axis(vl, sortidx, axis=1)  # [B, Q] ascending

    # slot s of core n holds batch 2n + s
    Ls_by_slot = [Ls[s::BPC] for s in range(BPC)]  # each [8, Q]
    struct = _compute_structure(Ls_by_slot)
    Ls_by_core_slot = [[Ls[n * BPC + s] for s in range(BPC)]
                       for n in range(N_CORES)]
    offsets, total_w, masks = _build_masks(struct, Ls_by_core_slot)

    key_sig = (total_w, tuple(
        (s, j, c, st, width, m_lo, m_w, last)
        for s in range(BPC) for j in range(NJ)
        for (c, st, width, m_lo, m_w, last) in struct[s][j]
    ))

    # ---- per-core input maps --------------------------------------------
    biases = (D / 2.0 - 0.5 * (keys.astype(np.float64) ** 2).sum(-1)) * ALPHA
    biases = biases.astype(np.float32)  # [B, K]

    in_maps = []
    for n in range(N_CORES):
        qt = np.empty((BPC, D, Q), np.float32)
        kt = np.empty((BPC, D, K), np.float32)
        vp = np.empty((BPC, 128, NKT * DV), np.float32)
        bias_arr = np.empty((BPC, 128, NKT), np.float32)
        for s in range(BPC):
            b = n * BPC + s
            qt[s] = queries[b][sortidx[b]].T
            kt[s] = keys[b].T
            vp[s] = (values[b].reshape(NKT, 128, DV)
                     .transpose(1, 0, 2).reshape(128, NKT * DV))
            bias_arr[s] = biases[b].reshape(NKT, 128).T
        in_maps.append({
            "qt": qt, "kt": kt, "vp": vp, "bias": bias_arr,
            "masks": np.ascontiguousarray(masks[n]),
            "ones": np.concatenate(
                [np.ones((128, 1), np.float32),
                 np.zeros((128, 128), np.float32)], axis=1),
        })
    return key_sig, struct, offsets, total_w, in_maps, sortidx


def get_program(key_sig, struct, offsets, total_w):
    if key_sig not in _program_cache:
        _program_cache.clear()
        _program_cache[key_sig] = _build_program(struct, offsets, total_w)
    return _program_cache[key_sig]


def kernel(queries, keys, values, valid_lens):
    global LAST_EXEC_NS, LAST_WALL_S, LAST_RESULTS
    key_sig, struct, offsets, total_w, in_maps, sortidx = _prepare(
        queries, keys, values, valid_lens
    )
    nc = get_program(key_sig, struct, offsets, total_w)

    # ---- run on 8 cores --------------------------------------------------
    from concourse.bass_utils import run_bass_kernel_spmd

    trace = bool(int(os.environ.get("KBENCH_TRACE", "0")))
    kwargs = {}
    tdir = os.environ.get("KBENCH_TRACE_DIR")
    if trace and tdir:
        kwargs["tmpdir"] = tdir
    t0 = time.perf_counter()
    try:
        res = run_bass_kernel_spmd(
            nc, in_maps, core_ids=list(range(N_CORES)), trace=trace, **kwargs
        )
    except Exception:
        if not trace:
            raise
        import traceback
        traceback.print_exc()
        res = run_bass_kernel_spmd(
            nc, in_maps, core_ids=list(range(N_CORES)), trace=False
        )
    LAST_WALL_S = time.perf_counter() - t0
    LAST_EXEC_NS = res.exec_time_ns
    LAST_RESULTS = res

    # ---- gather + undo the sort -----------------------------------------
    out = np.empty((B, Q, DV), dtype=np.float32)
    for n in range(N_CORES):
        o = res.results[n]["out"]  # [BPC, 128, (Q//128)*DV]
        for s in range(BPC):
            b = n * BPC + s
            # device rows: q_sorted = 128*t + p  ->  [p, t, d]
            osort = (o[s].reshape(128, Q // 128, DV)
                     .transpose(1, 0, 2).reshape(Q, DV))
            out[b][sortidx[b]] = osort
    return out
